# revision 15
# baseline (speedup 1.0000x reference)
"""HMM forward-algorithm log-likelihood kernel for Trainium2 (8 NeuronCores).

Problem: B=64 sequences, TMAX=2048 timesteps, N=256 hidden states, M=1024
emission symbols.  reference computes log p(x_b) via the log-domain forward
algorithm and gathers it at the last valid timestep T[b]-1.

Algorithm (validated to ~1e-4 rel against an fp64 oracle):
  *  Work in LINEAR space with the scaled forward recurrence
         v_{t} = Ehat[:, x_t] * (A @ v_{t-1})
     where A = softmax(trans, axis=0) (columns sum to 1) and
     Ehat = exp(log_softmax(emis,1) + lam) with a per-step scale e^lam chosen
     so log(sum v) stays near 0 (lam is calibrated at runtime on the host).
  *  Variable lengths: x is padded with an extra symbol (id M) whose emission
     column is exactly 1.0.  Since A is column-stochastic, padded steps
     preserve total mass, so logsumexp(alpha) freezes at the sequence end.
     Host corrects by T[b]*lam.
  *  Time-chunked parallel scan: sequences are split into chunks of C=16
     steps.  ONLY chunks that start before T[b] are computed (on random
     lengths that halves the work); the useful (seq, chunk) pairs are
     PACKED into a fixed per-core column count R at kernel() time (the
     program is compiled for that R and cached).  Each chunk is preceded
     by BURN=2 burn-in steps from the ones vector; the forward map
     contracts ~16x per step, so 2 steps push the direction error far
     below bf16 noise.  Per-chunk log-gains G = log(sum v_end) -
     log(sum v_start) telescope to the exact answer.
  *  Per core: R columns in two ping-pong groups of RG.  Per step/group:
     4 matmuls (256x256 A in 2x2 blocks of 128, free dim RG).  Emission
     columns are PRE-GATHERED ON THE HOST into a [NW, 128, W, 2, 2, RG]
     bf16 stream and double-buffered into SBUF with plain HWDGE DMA.
  *  PSUM: each (group, ic-half) accumulates into its OWN full 2KB bank.
  *  PSUM evacuation split across engines per group: the ic0 half (ready
     after 2 matmuls) goes ScalarE copy -> VectorE SBUF multiply; the ic1
     half (ready last) is multiplied directly out of PSUM on VectorE.
Output of the device kernel: per-core (1, 2, R) fp32 of column sums at
s=BURN (Zs) and s=STEPS (Ze).  Host combines gains per sequence, applies
the lam correction, and returns (64, 1) float32.
"""

import numpy as np
import ml_dtypes

import concourse.bass as bass
import concourse.bacc as bacc
import concourse.tile as tile
import concourse.mybir as mybir
import concourse.bass_utils as bass_utils

BF16 = ml_dtypes.bfloat16

# Problem constants (hardcoded; kernel.py must be self-contained).
B, TMAX, N, M = 64, 2048, 256, 1024
NCORES = 8
BLOC = B // NCORES          # 8 sequences per core (chunk-0 columns)

# Algorithm parameters.
C = 16                      # steps per chunk
BURN = 2                    # burn-in steps per chunk
STEPS = BURN + C            # 18 local steps
NGRP = 2                    # ping-pong groups (overlap PE with DVE/ACT)
W = 2                       # steps per DMA window (must divide STEPS)
NW = STEPS // W             # windows
NWARM = 12                  # PE warm-up matmuls issued while DMAs land

_CACHE = {}


def _log_softmax(a, axis):
    m = a.max(axis=axis, keepdims=True)
    s = a - m
    return s - np.log(np.exp(s).sum(axis=axis, keepdims=True))


def _build_program(R):
    """Build the SPMD Bass program (same NEFF for all 8 cores)."""
    RG = R // NGRP
    nc = bacc.Bacc(
        "TRN2",
        debug=False,
        enable_asserts=False,
        target_bir_lowering=False,
        num_devices=NCORES,
    )
    dt = mybir.dt

    at_d = nc.dram_tensor("at", [128, 2, 2, 128], dt.bfloat16, kind="ExternalInput")
    pi_d = nc.dram_tensor("pi0", [128, 2, BLOC], dt.bfloat16, kind="ExternalInput")
    # pre-gathered emission stream: [window, partition, step, group, ic, col]
    eg_d = nc.dram_tensor(
        "eg", [NW, 128, W, NGRP, 2, RG], dt.bfloat16, kind="ExternalInput"
    )
    zout_d = nc.dram_tensor("zout", [1, 2, R], dt.float32, kind="ExternalOutput")

    with tile.TileContext(nc) as tc:
        with (
            tc.tile_pool(name="singles", bufs=1) as singles,
            tc.tile_pool(name="state", bufs=1) as state,
            tc.tile_pool(name="eg", bufs=3) as egp,
            tc.tile_pool(name="work", bufs=2) as work,
            tc.tile_pool(name="ps", bufs=1, space="PSUM") as psp,
            tc.tile_pool(name="zps", bufs=2, space="PSUM") as zpsp,
        ):
            at_sb = singles.tile([128, 2, 2, 128], dt.bfloat16)
            pi_sb = singles.tile([128, 2, BLOC], dt.bfloat16)
            ones_sb = singles.tile([128, 1], dt.bfloat16)
            zbuf = singles.tile([1, 2, R], dt.float32)

            # state tiles, double-buffered by step parity
            v = [[None, None], [None, None]]  # v[g][parity]
            for g in range(NGRP):
                for par in range(2):
                    vt = state.tile(
                        [128, 2, RG], dt.bfloat16,
                        name=f"v{g}p{par}", tag=f"v{g}p{par}",
                    )
                    v[g][par] = vt

            # one full 2KB PSUM bank per (group, ic-half)
            ps = [[None, None], [None, None]]
            for g in range(NGRP):
                for ic in range(2):
                    ps[g][ic] = psp.tile(
                        [128, 512], dt.float32,
                        name=f"ps{g}{ic}", tag=f"ps{g}{ic}",
                    )

            def fetch(w):
                egt = egp.tile([128, W, NGRP, 2, RG], dt.bfloat16, tag="eg")
                nc.sync.dma_start(out=egt[:], in_=eg_d.ap()[w])
                return egt

            # small tables first (they gate the first matmul burst), then
            # two emission windows in flight
            nc.sync.dma_start(out=at_sb[:], in_=at_d.ap())
            nc.sync.dma_start(out=pi_sb[:], in_=pi_d.ap())
            nc.vector.memset(ones_sb[:], 1.0)
            nc.gpsimd.memset(v[0][0][:], 1.0)
            nc.gpsimd.memset(v[1][0][:], 1.0)
            egt = fetch(0)
            nxt1 = fetch(1) if NW > 1 else None

            # warm-up matmuls: keep the PE busy while the first emission
            # window lands so the HAM clock gate opens before step 1.
            # ones_sb x v000 -> scratch PSUM; no consumers.
            warm = zpsp.tile([1, RG], dt.float32, name="warm", tag="warm")
            for _ in range(NWARM):
                nc.tensor.matmul(
                    warm[:], ones_sb[:], v[0][0][:, 0, :],
                    start=True, stop=True,
                )

            def snapshot(ev, g, vt):
                zp = zpsp.tile([1, RG], dt.float32, tag="zps")
                nc.tensor.matmul(zp[:], ones_sb[:], vt[:, 0, :], start=True, stop=False)
                nc.tensor.matmul(zp[:], ones_sb[:], vt[:, 1, :], start=False, stop=True)
                nc.vector.tensor_copy(zbuf[:, ev, g * RG:(g + 1) * RG], zp[:])

            for w in range(NW):
                nxt2 = fetch(w + 2) if w + 2 < NW else None
                for sl in range(W):
                    s = w * W + sl + 1
                    pin = (s - 1) % 2
                    pout = s % 2
                    # weight-interleaved burst: each (kc, ic) weight is used by
                    # both groups back-to-back, halving LDWEIGHTS port pressure
                    for ic in range(2):
                        for kc in range(2):
                            for g in range(NGRP):
                                nc.tensor.matmul(
                                    ps[g][ic][:, 0:RG],
                                    at_sb[:, kc, ic, :],
                                    v[g][pin][:, kc, :],
                                    start=(kc == 0),
                                    stop=(kc == 1),
                                )
                    for g in range(NGRP):
                        vin = v[g][pin]
                        vout = v[g][pout]
                        # ic0: ScalarE evacuates PSUM, VectorE multiplies in SBUF
                        u = work.tile([128, RG], dt.bfloat16, tag=f"u{g}")
                        nc.scalar.activation(
                            u[:], ps[g][0][:, 0:RG],
                            mybir.ActivationFunctionType.Copy,
                        )
                        nc.vector.tensor_mul(
                            vout[:, 0, :], u[:], egt[:, sl, g, 0, :]
                        )
                        # ic1: VectorE multiplies straight out of PSUM
                        nc.vector.tensor_mul(
                            vout[:, 1, :], ps[g][1][:, 0:RG], egt[:, sl, g, 1, :]
                        )
                        if s == BURN and g == 0:
                            # chunk-0 columns sit at r = 0..BLOC-1 (group 0):
                            # overwrite with v_0 = Ehat[:, x[b,0]] * pi
                            nc.vector.tensor_mul(
                                vout[:, :, 0:BLOC],
                                egt[:, sl, 0, :, 0:BLOC],
                                pi_sb[:],
                            )
                        if s == BURN:
                            snapshot(0, g, vout)
                        if s == STEPS:
                            snapshot(1, g, vout)
                egt = nxt1
                nxt1 = nxt2
            nc.sync.dma_start(out=zout_d.ap(), in_=zbuf[:])

    nc.compile()
    return nc


def _pack_columns(T):
    """Pack useful (seq, chunk) pairs into per-core column lists.

    A chunk c of sequence b is useful iff c*C < T[b].  Chunk 0 of sequence
    b is pinned to core b // BLOC at column position b % BLOC (the device
    program applies the pi-init to columns 0..BLOC-1 of group 0).  The
    remaining useful chunks are distributed round-robin; pad columns get
    (b=0, c=-1) which the emission builder fills with the pad symbol.

    Returns (R, cols) with cols[core] a list of R (b, c) pairs (c == -1
    for padding).
    """
    useful = []
    for b in range(B):
        nch = int(min((int(T[b]) + C - 1) // C, TMAX // C))
        for c in range(1, nch):
            useful.append((b, c))
    U = B + len(useful)                      # chunk-0 columns + the rest
    R = max(2 * BLOC, -(-U // NCORES))
    R = ((R + 7) // 8) * 8                   # multiple of 8 (even RG, alignment)

    cols = [[] for _ in range(NCORES)]
    for b in range(B):
        cols[b // BLOC].append((b, 0))
    k = 0
    for core in range(NCORES):
        while len(cols[core]) < R and k < len(useful):
            cols[core].append(useful[k])
            k += 1
    # spill: if any core filled up before useful ran out, continue on others
    while k < len(useful):
        for core in range(NCORES):
            if len(cols[core]) < R and k < len(useful):
                cols[core].append(useful[k])
                k += 1
    for core in range(NCORES):
        while len(cols[core]) < R:
            cols[core].append((0, -1))       # pad column
    return R, cols


def _prep_inputs(x, T, pi, trans, emis):
    """Host preprocessing: tables, lambda calibration, pre-gathered emissions."""
    x = np.asarray(x).astype(np.int64)
    T = np.asarray(T).astype(np.int64)
    pi = np.asarray(pi, dtype=np.float64)
    trans = np.asarray(trans, dtype=np.float64)
    emis = np.asarray(emis, dtype=np.float64)

    log_pi = _log_softmax(pi, 0)
    log_A = _log_softmax(trans, 0)
    log_E = _log_softmax(emis, 1)
    pi_exp = np.exp(log_pi)
    A_exp = np.exp(log_A)

    # lambda calibration: short fp32 run of the normalized recurrence.
    Af = A_exp.astype(np.float32)
    Ef = np.exp(log_E).astype(np.float32)
    nseq = min(16, B)
    v = np.ones((N, nseq), dtype=np.float32) / N
    acc = []
    ncal = min(48, int(T.max()))
    for t in range(1, max(2, ncal)):
        sym = x[:nseq, t]
        w_ = Ef[:, sym] * (Af @ v)
        Z = w_.sum(axis=0)
        Z = np.maximum(Z, 1e-30)
        acc.append(np.log(Z))
        v = w_ / Z
    tail = acc[len(acc) // 3:]
    lam = -float(np.mean(np.concatenate(tail))) if tail else 7.0

    # Tables.
    # at[k, kc, ic, i] = A_exp[ic*128 + i, kc*128 + k]   (lhsT tiles)
    at = np.empty((128, 2, 2, 128), dtype=BF16)
    for kc in range(2):
        for ic in range(2):
            blk = A_exp[ic * 128:(ic + 1) * 128, kc * 128:(kc + 1) * 128]
            at[:, kc, ic, :] = blk.T.astype(BF16)
    # ehat rows: [m, i];  row M is all-ones (pad symbol)
    ehatT = np.ones((M + 1, N), dtype=BF16)
    ehatT[:M, :] = np.exp(log_E + lam).T.astype(BF16)
    # pi tile: [p, c, b] = pi_exp[c*128 + p]
    pi_t = np.empty((128, 2, BLOC), dtype=BF16)
    for c in range(2):
        pi_t[:, c, :] = np.repeat(
            pi_exp[c * 128:(c + 1) * 128].astype(BF16)[:, None], BLOC, axis=1
        )

    # padded x: t in [0, 2048]; pad symbol M for t >= T[b]
    x_pad = np.full((B, TMAX + 1), M, dtype=np.int64)
    x_pad[:, :TMAX] = x
    for b in range(B):
        x_pad[b, T[b]:] = M

    R, cols = _pack_columns(T)
    RG = R // NGRP

    # Symbol schedule per core: packed column r holds chunk (b, c);
    # local step s applies transition t = c*C - BURN + s.
    # t out of range or pad column -> pad symbol; (c == 0, s == BURN) ->
    # x[b, 0] (init overwrite).
    s_arr = np.arange(1, STEPS + 1)[:, None]          # (STEPS, 1)
    eg_tensors = []
    for core in range(NCORES):
        bc = np.array(cols[core], dtype=np.int64)     # (R, 2)
        b_arr = bc[None, :, 0]                        # (1, R)
        c_arr = bc[None, :, 1]                        # (1, R)
        t_arr = c_arr * C - BURN + s_arr              # (STEPS, R)
        sym = np.where(
            (c_arr < 0) | (t_arr < 1) | (t_arr > TMAX),
            M,
            x_pad[np.broadcast_to(b_arr, t_arr.shape),
                  np.clip(t_arr, 1, TMAX)],
        )
        init_mask = (c_arr == 0) & (s_arr == BURN)
        sym = np.where(
            init_mask, x_pad[np.broadcast_to(b_arr, t_arr.shape), 0], sym
        )
        # big[s, r, n] -> eg[nw, p, sl, g, ic, rg]
        big = ehatT[sym]                              # (STEPS, R, N) bf16
        eg = big.reshape(NW, W, NGRP, RG, 2, 128).transpose(0, 5, 1, 2, 4, 3)
        eg_tensors.append(np.ascontiguousarray(eg))

    host = {
        "lam": lam,
        "T": T,
        "R": R,
        "cols": cols,
        "at": np.ascontiguousarray(at),
        "pi_t": np.ascontiguousarray(pi_t),
        "eg": eg_tensors,
    }
    return host


def _postprocess(zouts, host):
    """Combine per-core (1, 2, R) Zs/Ze into (B, 1) float32 log-probs."""
    lam, T, R, cols = host["lam"], host["T"], host["R"], host["cols"]
    Gsum = np.zeros(B, dtype=np.float64)
    L0 = np.zeros(B, dtype=np.float64)
    for core in range(NCORES):
        z = np.asarray(zouts[core], dtype=np.float64).reshape(2, R)
        Zs, Ze = z[0], z[1]
        with np.errstate(divide="ignore", invalid="ignore"):
            G = np.log(Ze) - np.log(Zs)
        for r, (b, c) in enumerate(cols[core]):
            if c < 0:
                continue
            Gsum[b] += G[r]
            if c == 0:
                L0[b] = np.log(Zs[r])
    L = L0 + Gsum - T * lam
    return L.reshape(B, 1).astype(np.float32)


def _make_in_maps(host):
    in_maps = []
    for core in range(NCORES):
        in_maps.append(
            {
                "at": host["at"],
                "pi0": host["pi_t"],
                "eg": host["eg"][core],
            }
        )
    return in_maps


def kernel(x, T, pi, trans, emis):
    host = _prep_inputs(x, T, pi, trans, emis)

    key = ("nc", host["R"])
    if key not in _CACHE:
        _CACHE[key] = _build_program(host["R"])
    nc = _CACHE[key]

    res = bass_utils.run_bass_kernel_spmd(
        nc, _make_in_maps(host), core_ids=list(range(NCORES))
    )
    zouts = [r["zout"] for r in res.results]
    return _postprocess(zouts, host)


def profile(inputs, tmpdir=None):
    """Run with trace=True; returns max-across-cores exec_time_ns."""
    host = _prep_inputs(**inputs)
    key = ("nc", host["R"])
    if key not in _CACHE:
        _CACHE[key] = _build_program(host["R"])
    nc = _CACHE[key]
    res = bass_utils.run_bass_kernel_spmd(
        nc,
        _make_in_maps(host),
        core_ids=list(range(NCORES)),
        trace=True,
        tmpdir=tmpdir,
    )
    return res.exec_time_ns


# revision 16
# speedup vs baseline: 1.1530x; 1.1530x over previous
"""HMM forward-algorithm log-likelihood kernel for Trainium2 (8 NeuronCores).

Problem: B=64 sequences, TMAX=2048 timesteps, N=256 hidden states, M=1024
emission symbols.  reference computes log p(x_b) via the log-domain forward
algorithm and gathers it at the last valid timestep T[b]-1.

Algorithm (validated to ~1e-4 rel against an fp64 oracle):
  *  Work in LINEAR space with the scaled forward recurrence
         v_{t} = Ehat[:, x_t] * (A @ v_{t-1})
     where A = softmax(trans, axis=0) (columns sum to 1) and
     Ehat = exp(log_softmax(emis,1) + lam) with a per-step scale e^lam chosen
     so log(sum v) stays near 0 (lam is calibrated at runtime on the host).
  *  Variable lengths: x is padded with an extra symbol (id M) whose emission
     column is exactly 1.0.  Since A is column-stochastic, padded steps
     preserve total mass, so logsumexp(alpha) freezes at the sequence end.
     Host corrects by T[b]*lam.
  *  Time-chunked parallel scan: sequences are split into chunks of C=16
     steps.  ONLY chunks that start before T[b] are computed (on random
     lengths that halves the work); the useful (seq, chunk) pairs are
     PACKED into a fixed per-core column count R at kernel() time (the
     program is compiled for that R and cached).  Each chunk is preceded
     by BURN=2 burn-in steps from the ones vector; the forward map
     contracts ~16x per step, so 2 steps push the direction error far
     below bf16 noise.  Per-chunk log-gains G = log(sum v_end) -
     log(sum v_start) telescope to the exact answer.
  *  Per core: R columns in two ping-pong groups of RG.  Per step/group:
     4 matmuls (256x256 A in 2x2 blocks of 128, free dim RG).  Emission
     columns are PRE-GATHERED ON THE HOST into a [NW, 128, W, 2, 2, RG]
     bf16 stream and double-buffered into SBUF with plain HWDGE DMA.
  *  PSUM: each (group, ic-half) accumulates into its OWN full 2KB bank.
  *  PSUM evacuation split across engines per group: the ic0 half (ready
     after 2 matmuls) goes ScalarE copy -> VectorE SBUF multiply; the ic1
     half (ready last) is multiplied directly out of PSUM on VectorE.
Output of the device kernel: per-core (1, 2, R) fp32 of column sums at
s=BURN (Zs) and s=STEPS (Ze).  Host combines gains per sequence, applies
the lam correction, and returns (64, 1) float32.
"""

import numpy as np
import ml_dtypes

import concourse.bass as bass
import concourse.bacc as bacc
import concourse.tile as tile
import concourse.mybir as mybir
import concourse.bass_utils as bass_utils

BF16 = ml_dtypes.bfloat16

# Problem constants (hardcoded; kernel.py must be self-contained).
B, TMAX, N, M = 64, 2048, 256, 1024
NCORES = 8
BLOC = B // NCORES          # 8 sequences per core (chunk-0 columns)

# Algorithm parameters.
C = 16                      # steps per chunk
BURN = 2                    # burn-in steps per chunk
STEPS = BURN + C            # 18 local steps
NGRP = 2                    # ping-pong groups (overlap PE with DVE/ACT)
W = 2                       # steps per DMA window (must divide STEPS)
NW = STEPS // W             # windows
NWARM = 12                  # PE warm-up matmuls issued while DMAs land

_CACHE = {}


def _log_softmax(a, axis):
    m = a.max(axis=axis, keepdims=True)
    s = a - m
    return s - np.log(np.exp(s).sum(axis=axis, keepdims=True))


def _build_program(R):
    """Build the SPMD Bass program (same NEFF for all 8 cores)."""
    RG = R // NGRP
    nc = bacc.Bacc(
        "TRN2",
        debug=False,
        enable_asserts=False,
        target_bir_lowering=False,
        num_devices=NCORES,
    )
    dt = mybir.dt

    at_d = nc.dram_tensor("at", [128, 2, 2, 128], dt.bfloat16, kind="ExternalInput")
    pi_d = nc.dram_tensor("pi0", [128, 2, BLOC], dt.bfloat16, kind="ExternalInput")
    # pre-gathered emission stream: [window, partition, step, group, ic, col]
    eg_d = nc.dram_tensor(
        "eg", [NW, 128, W, NGRP, 2, RG], dt.bfloat16, kind="ExternalInput"
    )
    zout_d = nc.dram_tensor("zout", [1, 2, R], dt.float32, kind="ExternalOutput")

    with tile.TileContext(nc) as tc:
        with (
            tc.tile_pool(name="singles", bufs=1) as singles,
            tc.tile_pool(name="state", bufs=1) as state,
            tc.tile_pool(name="eg", bufs=3) as egp,
            tc.tile_pool(name="work", bufs=2) as work,
            tc.tile_pool(name="ps", bufs=1, space="PSUM") as psp,
            tc.tile_pool(name="zps", bufs=2, space="PSUM") as zpsp,
        ):
            at_sb = singles.tile([128, 2, 2, 128], dt.bfloat16)
            pi_sb = singles.tile([128, 2, BLOC], dt.bfloat16)
            ones_sb = singles.tile([128, 1], dt.bfloat16)
            zbuf = singles.tile([1, 2, R], dt.float32)

            # state tiles, double-buffered by step parity
            v = [[None, None], [None, None]]  # v[g][parity]
            for g in range(NGRP):
                for par in range(2):
                    vt = state.tile(
                        [128, 2, RG], dt.bfloat16,
                        name=f"v{g}p{par}", tag=f"v{g}p{par}",
                    )
                    v[g][par] = vt

            # one full 2KB PSUM bank per (group, ic-half)
            ps = [[None, None], [None, None]]
            for g in range(NGRP):
                for ic in range(2):
                    ps[g][ic] = psp.tile(
                        [128, 512], dt.float32,
                        name=f"ps{g}{ic}", tag=f"ps{g}{ic}",
                    )

            def fetch(w):
                egt = egp.tile([128, W, NGRP, 2, RG], dt.bfloat16, tag="eg")
                nc.sync.dma_start(out=egt[:], in_=eg_d.ap()[w])
                return egt

            # small tables first (they gate the first matmul burst), then
            # two emission windows in flight
            nc.sync.dma_start(out=at_sb[:], in_=at_d.ap())
            nc.sync.dma_start(out=pi_sb[:], in_=pi_d.ap())
            nc.vector.memset(ones_sb[:], 1.0)
            nc.gpsimd.memset(v[0][0][:], 1.0)
            nc.gpsimd.memset(v[1][0][:], 1.0)
            egt = fetch(0)
            nxt1 = fetch(1) if NW > 1 else None

            # warm-up matmuls: keep the PE busy while the first emission
            # window lands so the HAM clock gate opens before step 1.
            # ones_sb x v000 -> scratch PSUM; no consumers.
            warm = zpsp.tile([1, RG], dt.float32, name="warm", tag="warm")
            for _ in range(NWARM):
                nc.tensor.matmul(
                    warm[:], ones_sb[:], v[0][0][:, 0, :],
                    start=True, stop=True,
                )

            def snapshot(ev, g, vt):
                zp = zpsp.tile([1, RG], dt.float32, tag="zps")
                nc.tensor.matmul(zp[:], ones_sb[:], vt[:, 0, :], start=True, stop=False)
                nc.tensor.matmul(zp[:], ones_sb[:], vt[:, 1, :], start=False, stop=True)
                nc.vector.tensor_copy(zbuf[:, ev, g * RG:(g + 1) * RG], zp[:])

            for w in range(NW):
                nxt2 = fetch(w + 2) if w + 2 < NW else None
                for sl in range(W):
                    s = w * W + sl + 1
                    pin = (s - 1) % 2
                    pout = s % 2
                    for g in range(NGRP):
                        vin = v[g][pin]
                        vout = v[g][pout]
                        # ic-major matmul order: ic0 ready after 2 MMs
                        for ic in range(2):
                            for kc in range(2):
                                nc.tensor.matmul(
                                    ps[g][ic][:, 0:RG],
                                    at_sb[:, kc, ic, :],
                                    vin[:, kc, :],
                                    start=(kc == 0),
                                    stop=(kc == 1),
                                )
                        # ic0: ScalarE evacuates PSUM, VectorE multiplies in SBUF
                        u = work.tile([128, RG], dt.bfloat16, tag=f"u{g}")
                        nc.scalar.activation(
                            u[:], ps[g][0][:, 0:RG],
                            mybir.ActivationFunctionType.Copy,
                        )
                        nc.vector.tensor_mul(
                            vout[:, 0, :], u[:], egt[:, sl, g, 0, :]
                        )
                        # ic1: VectorE multiplies straight out of PSUM
                        nc.vector.tensor_mul(
                            vout[:, 1, :], ps[g][1][:, 0:RG], egt[:, sl, g, 1, :]
                        )
                        if s == BURN and g == 0:
                            # chunk-0 columns sit at r = 0..BLOC-1 (group 0):
                            # overwrite with v_0 = Ehat[:, x[b,0]] * pi
                            nc.vector.tensor_mul(
                                vout[:, :, 0:BLOC],
                                egt[:, sl, 0, :, 0:BLOC],
                                pi_sb[:],
                            )
                        if s == BURN:
                            snapshot(0, g, vout)
                        if s == STEPS:
                            snapshot(1, g, vout)
                egt = nxt1
                nxt1 = nxt2
            nc.sync.dma_start(out=zout_d.ap(), in_=zbuf[:])

    nc.compile()
    return nc


def _pack_columns(T):
    """Pack useful (seq, chunk) pairs into per-core column lists.

    A chunk c of sequence b is useful iff c*C < T[b].  Chunk 0 of sequence
    b is pinned to core b // BLOC at column position b % BLOC (the device
    program applies the pi-init to columns 0..BLOC-1 of group 0).  The
    remaining useful chunks are distributed round-robin; pad columns get
    (b=0, c=-1) which the emission builder fills with the pad symbol.

    Returns (R, cols) with cols[core] a list of R (b, c) pairs (c == -1
    for padding).
    """
    useful = []
    for b in range(B):
        nch = int(min((int(T[b]) + C - 1) // C, TMAX // C))
        for c in range(1, nch):
            useful.append((b, c))
    U = B + len(useful)                      # chunk-0 columns + the rest
    R = max(2 * BLOC, -(-U // NCORES))
    R = ((R + 7) // 8) * 8                   # multiple of 8 (even RG, alignment)

    cols = [[] for _ in range(NCORES)]
    for b in range(B):
        cols[b // BLOC].append((b, 0))
    k = 0
    for core in range(NCORES):
        while len(cols[core]) < R and k < len(useful):
            cols[core].append(useful[k])
            k += 1
    # spill: if any core filled up before useful ran out, continue on others
    while k < len(useful):
        for core in range(NCORES):
            if len(cols[core]) < R and k < len(useful):
                cols[core].append(useful[k])
                k += 1
    for core in range(NCORES):
        while len(cols[core]) < R:
            cols[core].append((0, -1))       # pad column
    return R, cols


def _prep_inputs(x, T, pi, trans, emis):
    """Host preprocessing: tables, lambda calibration, pre-gathered emissions."""
    x = np.asarray(x).astype(np.int64)
    T = np.asarray(T).astype(np.int64)
    pi = np.asarray(pi, dtype=np.float64)
    trans = np.asarray(trans, dtype=np.float64)
    emis = np.asarray(emis, dtype=np.float64)

    log_pi = _log_softmax(pi, 0)
    log_A = _log_softmax(trans, 0)
    log_E = _log_softmax(emis, 1)
    pi_exp = np.exp(log_pi)
    A_exp = np.exp(log_A)

    # lambda calibration: short fp32 run of the normalized recurrence.
    Af = A_exp.astype(np.float32)
    Ef = np.exp(log_E).astype(np.float32)
    nseq = min(16, B)
    v = np.ones((N, nseq), dtype=np.float32) / N
    acc = []
    ncal = min(48, int(T.max()))
    for t in range(1, max(2, ncal)):
        sym = x[:nseq, t]
        w_ = Ef[:, sym] * (Af @ v)
        Z = w_.sum(axis=0)
        Z = np.maximum(Z, 1e-30)
        acc.append(np.log(Z))
        v = w_ / Z
    tail = acc[len(acc) // 3:]
    lam = -float(np.mean(np.concatenate(tail))) if tail else 7.0

    # Tables.
    # at[k, kc, ic, i] = A_exp[ic*128 + i, kc*128 + k]   (lhsT tiles)
    at = np.empty((128, 2, 2, 128), dtype=BF16)
    for kc in range(2):
        for ic in range(2):
            blk = A_exp[ic * 128:(ic + 1) * 128, kc * 128:(kc + 1) * 128]
            at[:, kc, ic, :] = blk.T.astype(BF16)
    # ehat rows: [m, i];  row M is all-ones (pad symbol)
    ehatT = np.ones((M + 1, N), dtype=BF16)
    ehatT[:M, :] = np.exp(log_E + lam).T.astype(BF16)
    # pi tile: [p, c, b] = pi_exp[c*128 + p]
    pi_t = np.empty((128, 2, BLOC), dtype=BF16)
    for c in range(2):
        pi_t[:, c, :] = np.repeat(
            pi_exp[c * 128:(c + 1) * 128].astype(BF16)[:, None], BLOC, axis=1
        )

    # padded x: t in [0, 2048]; pad symbol M for t >= T[b]
    x_pad = np.full((B, TMAX + 1), M, dtype=np.int64)
    x_pad[:, :TMAX] = x
    for b in range(B):
        x_pad[b, T[b]:] = M

    R, cols = _pack_columns(T)
    RG = R // NGRP

    # Symbol schedule per core: packed column r holds chunk (b, c);
    # local step s applies transition t = c*C - BURN + s.
    # t out of range or pad column -> pad symbol; (c == 0, s == BURN) ->
    # x[b, 0] (init overwrite).
    s_arr = np.arange(1, STEPS + 1)[:, None]          # (STEPS, 1)
    eg_tensors = []
    for core in range(NCORES):
        bc = np.array(cols[core], dtype=np.int64)     # (R, 2)
        b_arr = bc[None, :, 0]                        # (1, R)
        c_arr = bc[None, :, 1]                        # (1, R)
        t_arr = c_arr * C - BURN + s_arr              # (STEPS, R)
        sym = np.where(
            (c_arr < 0) | (t_arr < 1) | (t_arr > TMAX),
            M,
            x_pad[np.broadcast_to(b_arr, t_arr.shape),
                  np.clip(t_arr, 1, TMAX)],
        )
        init_mask = (c_arr == 0) & (s_arr == BURN)
        sym = np.where(
            init_mask, x_pad[np.broadcast_to(b_arr, t_arr.shape), 0], sym
        )
        # big[s, r, n] -> eg[nw, p, sl, g, ic, rg]
        big = ehatT[sym]                              # (STEPS, R, N) bf16
        eg = big.reshape(NW, W, NGRP, RG, 2, 128).transpose(0, 5, 1, 2, 4, 3)
        eg_tensors.append(np.ascontiguousarray(eg))

    host = {
        "lam": lam,
        "T": T,
        "R": R,
        "cols": cols,
        "at": np.ascontiguousarray(at),
        "pi_t": np.ascontiguousarray(pi_t),
        "eg": eg_tensors,
    }
    return host


def _postprocess(zouts, host):
    """Combine per-core (1, 2, R) Zs/Ze into (B, 1) float32 log-probs."""
    lam, T, R, cols = host["lam"], host["T"], host["R"], host["cols"]
    Gsum = np.zeros(B, dtype=np.float64)
    L0 = np.zeros(B, dtype=np.float64)
    for core in range(NCORES):
        z = np.asarray(zouts[core], dtype=np.float64).reshape(2, R)
        Zs, Ze = z[0], z[1]
        with np.errstate(divide="ignore", invalid="ignore"):
            G = np.log(Ze) - np.log(Zs)
        for r, (b, c) in enumerate(cols[core]):
            if c < 0:
                continue
            Gsum[b] += G[r]
            if c == 0:
                L0[b] = np.log(Zs[r])
    L = L0 + Gsum - T * lam
    return L.reshape(B, 1).astype(np.float32)


def _make_in_maps(host):
    in_maps = []
    for core in range(NCORES):
        in_maps.append(
            {
                "at": host["at"],
                "pi0": host["pi_t"],
                "eg": host["eg"][core],
            }
        )
    return in_maps


def kernel(x, T, pi, trans, emis):
    host = _prep_inputs(x, T, pi, trans, emis)

    key = ("nc", host["R"])
    if key not in _CACHE:
        _CACHE[key] = _build_program(host["R"])
    nc = _CACHE[key]

    res = bass_utils.run_bass_kernel_spmd(
        nc, _make_in_maps(host), core_ids=list(range(NCORES))
    )
    zouts = [r["zout"] for r in res.results]
    return _postprocess(zouts, host)


def profile(inputs, tmpdir=None):
    """Run with trace=True; returns max-across-cores exec_time_ns."""
    host = _prep_inputs(**inputs)
    key = ("nc", host["R"])
    if key not in _CACHE:
        _CACHE[key] = _build_program(host["R"])
    nc = _CACHE[key]
    res = bass_utils.run_bass_kernel_spmd(
        nc,
        _make_in_maps(host),
        core_ids=list(range(NCORES)),
        trace=True,
        tmpdir=tmpdir,
    )
    return res.exec_time_ns


# revision 17
# speedup vs baseline: 1.1936x; 1.0352x over previous
"""HMM forward-algorithm log-likelihood kernel for Trainium2 (8 NeuronCores).

Problem: B=64 sequences, TMAX=2048 timesteps, N=256 hidden states, M=1024
emission symbols.  reference computes log p(x_b) via the log-domain forward
algorithm and gathers it at the last valid timestep T[b]-1.

Algorithm (validated to ~1e-4 rel against an fp64 oracle):
  *  Work in LINEAR space with the scaled forward recurrence
         v_{t} = Ehat[:, x_t] * (A @ v_{t-1})
     where A = softmax(trans, axis=0) (columns sum to 1) and
     Ehat = exp(log_softmax(emis,1) + lam) with a per-step scale e^lam chosen
     so log(sum v) stays near 0 (lam is calibrated at runtime on the host).
  *  Variable lengths: x is padded with an extra symbol (id M) whose emission
     column is exactly 1.0.  Since A is column-stochastic, padded steps
     preserve total mass, so logsumexp(alpha) freezes at the sequence end.
     Host corrects by T[b]*lam.
  *  Time-chunked parallel scan: sequences are split into chunks of C=16
     steps.  ONLY chunks that start before T[b] are computed (on random
     lengths that halves the work); the useful (seq, chunk) pairs are
     PACKED into a fixed per-core column count R at kernel() time (the
     program is compiled for that R and cached).  Each chunk is preceded
     by BURN=2 burn-in steps from the ones vector; the forward map
     contracts ~16x per step, so 2 steps push the direction error far
     below bf16 noise.  Per-chunk log-gains G = log(sum v_end) -
     log(sum v_start) telescope to the exact answer.
  *  Per core: R columns in two ping-pong groups of RG.  Per step/group:
     4 matmuls (256x256 A in 2x2 blocks of 128, free dim RG).  Emission
     columns are PRE-GATHERED ON THE HOST into a [NW, 128, W, 2, 2, RG]
     bf16 stream and double-buffered into SBUF with plain HWDGE DMA.
  *  PSUM: each (group, ic-half) accumulates into its OWN full 2KB bank.
  *  PSUM evacuation split across engines per group: the ic0 half (ready
     after 2 matmuls) goes ScalarE copy -> VectorE SBUF multiply; the ic1
     half (ready last) is multiplied directly out of PSUM on VectorE.
Output of the device kernel: per-core (1, 2, R) fp32 of column sums at
s=BURN (Zs) and s=STEPS (Ze).  Host combines gains per sequence, applies
the lam correction, and returns (64, 1) float32.
"""

import numpy as np
import ml_dtypes

import concourse.bass as bass
import concourse.bacc as bacc
import concourse.tile as tile
import concourse.mybir as mybir
import concourse.bass_utils as bass_utils

BF16 = ml_dtypes.bfloat16

# Problem constants (hardcoded; kernel.py must be self-contained).
B, TMAX, N, M = 64, 2048, 256, 1024
NCORES = 8
BLOC = B // NCORES          # 8 sequences per core (chunk-0 columns)

# Algorithm parameters.
C = 16                      # steps per chunk
BURN = 2                    # burn-in steps per chunk
STEPS = BURN + C            # 18 local steps
NGRP = 2                    # ping-pong groups (overlap PE with DVE/ACT)
W = 2                       # steps per DMA window (must divide STEPS)
NW = STEPS // W             # windows
NWARM = 12                  # PE warm-up matmuls issued while DMAs land

_CACHE = {}


def _log_softmax(a, axis):
    m = a.max(axis=axis, keepdims=True)
    s = a - m
    return s - np.log(np.exp(s).sum(axis=axis, keepdims=True))


def _build_program(R):
    """Build the SPMD Bass program (same NEFF for all 8 cores)."""
    RG = R // NGRP
    nc = bacc.Bacc(
        "TRN2",
        debug=False,
        enable_asserts=False,
        target_bir_lowering=False,
        num_devices=NCORES,
    )
    dt = mybir.dt

    at_d = nc.dram_tensor("at", [128, 2, 2, 128], dt.bfloat16, kind="ExternalInput")
    pi_d = nc.dram_tensor("pi0", [128, 2, BLOC], dt.bfloat16, kind="ExternalInput")
    # pre-gathered emission stream: [window, partition, step, group, ic, col]
    eg_d = nc.dram_tensor(
        "eg", [NW, 128, W, NGRP, 2, RG], dt.bfloat16, kind="ExternalInput"
    )
    zout_d = nc.dram_tensor("zout", [1, 2, R], dt.float32, kind="ExternalOutput")

    with tile.TileContext(nc) as tc:
        with (
            tc.tile_pool(name="singles", bufs=1) as singles,
            tc.tile_pool(name="state", bufs=1) as state,
            tc.tile_pool(name="eg", bufs=3) as egp,
            tc.tile_pool(name="work", bufs=2) as work,
            tc.tile_pool(name="ps", bufs=1, space="PSUM") as psp,
            tc.tile_pool(name="zps", bufs=2, space="PSUM") as zpsp,
        ):
            at_sb = singles.tile([128, 2, 2, 128], dt.bfloat16)
            pi_sb = singles.tile([128, 2, BLOC], dt.bfloat16)
            ones_sb = singles.tile([128, 1], dt.bfloat16)
            zbuf = singles.tile([1, 2, R], dt.float32)

            # state tiles, double-buffered by step parity
            v = [[None, None], [None, None]]  # v[g][parity]
            for g in range(NGRP):
                for par in range(2):
                    vt = state.tile(
                        [128, 2, RG], dt.bfloat16,
                        name=f"v{g}p{par}", tag=f"v{g}p{par}",
                    )
                    v[g][par] = vt

            # one full 2KB PSUM bank per (group, ic-half)
            ps = [[None, None], [None, None]]
            for g in range(NGRP):
                for ic in range(2):
                    ps[g][ic] = psp.tile(
                        [128, 512], dt.float32,
                        name=f"ps{g}{ic}", tag=f"ps{g}{ic}",
                    )

            def fetch(w):
                egt = egp.tile([128, W, NGRP, 2, RG], dt.bfloat16, tag="eg")
                nc.sync.dma_start(out=egt[:], in_=eg_d.ap()[w])
                return egt

            # small tables first (they gate the first matmul burst), then
            # two emission windows in flight
            nc.sync.dma_start(out=at_sb[:], in_=at_d.ap())
            nc.sync.dma_start(out=pi_sb[:], in_=pi_d.ap())
            nc.vector.memset(ones_sb[:], 1.0)
            nc.gpsimd.memset(v[0][0][:], 1.0)
            nc.gpsimd.memset(v[1][0][:], 1.0)
            egt = fetch(0)
            nxt1 = fetch(1) if NW > 1 else None

            # warm-up matmuls: keep the PE busy while the first emission
            # window lands so the HAM clock gate opens before step 1.
            # ones_sb x v000 -> scratch PSUM; no consumers.
            warm = zpsp.tile([1, RG], dt.float32, name="warm", tag="warm")
            for _ in range(NWARM):
                nc.tensor.matmul(
                    warm[:], ones_sb[:], v[0][0][:, 0, :],
                    start=True, stop=True,
                )

            def snapshot(ev, g, vt):
                zp = zpsp.tile([1, RG], dt.float32, tag="zps")
                nc.tensor.matmul(zp[:], ones_sb[:], vt[:, 0, :], start=True, stop=False)
                nc.tensor.matmul(zp[:], ones_sb[:], vt[:, 1, :], start=False, stop=True)
                nc.vector.tensor_copy(zbuf[:, ev, g * RG:(g + 1) * RG], zp[:])

            for w in range(NW):
                nxt2 = fetch(w + 2) if w + 2 < NW else None
                for sl in range(W):
                    s = w * W + sl + 1
                    pin = (s - 1) % 2
                    pout = s % 2
                    for g in range(NGRP):
                        vin = v[g][pin]
                        vout = v[g][pout]
                        # ic-major matmul order: ic0 ready after 2 MMs.
                        # kc1 leads each pair so the burst's first matmul
                        # consumes the direct-path vt half (produced first
                        # on VectorE); the evac-path half is only needed
                        # one matmul later.
                        for ic in range(2):
                            for kc in (1, 0):
                                nc.tensor.matmul(
                                    ps[g][ic][:, 0:RG],
                                    at_sb[:, kc, ic, :],
                                    vin[:, kc, :],
                                    start=(kc == 1),
                                    stop=(kc == 0),
                                )
                        # ic0: ScalarE evacuates PSUM -> VectorE SBUF multiply
                        u = work.tile([128, RG], dt.bfloat16, tag=f"u{g}")
                        nc.scalar.activation(
                            u[:], ps[g][0][:, 0:RG],
                            mybir.ActivationFunctionType.Copy,
                        )
                        # ic1: VectorE multiplies straight out of PSUM; emitted
                        # first so it drains ahead of the SBUF multiply
                        nc.vector.tensor_mul(
                            vout[:, 1, :], ps[g][1][:, 0:RG], egt[:, sl, g, 1, :]
                        )
                        nc.vector.tensor_mul(
                            vout[:, 0, :], u[:], egt[:, sl, g, 0, :]
                        )
                        if s == BURN and g == 0:
                            # chunk-0 columns sit at r = 0..BLOC-1 (group 0):
                            # overwrite with v_0 = Ehat[:, x[b,0]] * pi
                            nc.vector.tensor_mul(
                                vout[:, :, 0:BLOC],
                                egt[:, sl, 0, :, 0:BLOC],
                                pi_sb[:],
                            )
                        if s == BURN:
                            snapshot(0, g, vout)
                        if s == STEPS:
                            snapshot(1, g, vout)
                egt = nxt1
                nxt1 = nxt2
            nc.sync.dma_start(out=zout_d.ap(), in_=zbuf[:])

    nc.compile()
    return nc


def _pack_columns(T):
    """Pack useful (seq, chunk) pairs into per-core column lists.

    A chunk c of sequence b is useful iff c*C < T[b].  Chunk 0 of sequence
    b is pinned to core b // BLOC at column position b % BLOC (the device
    program applies the pi-init to columns 0..BLOC-1 of group 0).  The
    remaining useful chunks are distributed round-robin; pad columns get
    (b=0, c=-1) which the emission builder fills with the pad symbol.

    Returns (R, cols) with cols[core] a list of R (b, c) pairs (c == -1
    for padding).
    """
    useful = []
    for b in range(B):
        nch = int(min((int(T[b]) + C - 1) // C, TMAX // C))
        for c in range(1, nch):
            useful.append((b, c))
    U = B + len(useful)                      # chunk-0 columns + the rest
    R = max(2 * BLOC, -(-U // NCORES))
    R = ((R + 7) // 8) * 8                   # multiple of 8 (even RG, alignment)

    cols = [[] for _ in range(NCORES)]
    for b in range(B):
        cols[b // BLOC].append((b, 0))
    k = 0
    for core in range(NCORES):
        while len(cols[core]) < R and k < len(useful):
            cols[core].append(useful[k])
            k += 1
    # spill: if any core filled up before useful ran out, continue on others
    while k < len(useful):
        for core in range(NCORES):
            if len(cols[core]) < R and k < len(useful):
                cols[core].append(useful[k])
                k += 1
    for core in range(NCORES):
        while len(cols[core]) < R:
            cols[core].append((0, -1))       # pad column
    return R, cols


def _prep_inputs(x, T, pi, trans, emis):
    """Host preprocessing: tables, lambda calibration, pre-gathered emissions."""
    x = np.asarray(x).astype(np.int64)
    T = np.asarray(T).astype(np.int64)
    pi = np.asarray(pi, dtype=np.float64)
    trans = np.asarray(trans, dtype=np.float64)
    emis = np.asarray(emis, dtype=np.float64)

    log_pi = _log_softmax(pi, 0)
    log_A = _log_softmax(trans, 0)
    log_E = _log_softmax(emis, 1)
    pi_exp = np.exp(log_pi)
    A_exp = np.exp(log_A)

    # lambda calibration: short fp32 run of the normalized recurrence.
    Af = A_exp.astype(np.float32)
    Ef = np.exp(log_E).astype(np.float32)
    nseq = min(16, B)
    v = np.ones((N, nseq), dtype=np.float32) / N
    acc = []
    ncal = min(48, int(T.max()))
    for t in range(1, max(2, ncal)):
        sym = x[:nseq, t]
        w_ = Ef[:, sym] * (Af @ v)
        Z = w_.sum(axis=0)
        Z = np.maximum(Z, 1e-30)
        acc.append(np.log(Z))
        v = w_ / Z
    tail = acc[len(acc) // 3:]
    lam = -float(np.mean(np.concatenate(tail))) if tail else 7.0

    # Tables.
    # at[k, kc, ic, i] = A_exp[ic*128 + i, kc*128 + k]   (lhsT tiles)
    at = np.empty((128, 2, 2, 128), dtype=BF16)
    for kc in range(2):
        for ic in range(2):
            blk = A_exp[ic * 128:(ic + 1) * 128, kc * 128:(kc + 1) * 128]
            at[:, kc, ic, :] = blk.T.astype(BF16)
    # ehat rows: [m, i];  row M is all-ones (pad symbol)
    ehatT = np.ones((M + 1, N), dtype=BF16)
    ehatT[:M, :] = np.exp(log_E + lam).T.astype(BF16)
    # pi tile: [p, c, b] = pi_exp[c*128 + p]
    pi_t = np.empty((128, 2, BLOC), dtype=BF16)
    for c in range(2):
        pi_t[:, c, :] = np.repeat(
            pi_exp[c * 128:(c + 1) * 128].astype(BF16)[:, None], BLOC, axis=1
        )

    # padded x: t in [0, 2048]; pad symbol M for t >= T[b]
    x_pad = np.full((B, TMAX + 1), M, dtype=np.int64)
    x_pad[:, :TMAX] = x
    for b in range(B):
        x_pad[b, T[b]:] = M

    R, cols = _pack_columns(T)
    RG = R // NGRP

    # Symbol schedule per core: packed column r holds chunk (b, c);
    # local step s applies transition t = c*C - BURN + s.
    # t out of range or pad column -> pad symbol; (c == 0, s == BURN) ->
    # x[b, 0] (init overwrite).
    s_arr = np.arange(1, STEPS + 1)[:, None]          # (STEPS, 1)
    eg_tensors = []
    for core in range(NCORES):
        bc = np.array(cols[core], dtype=np.int64)     # (R, 2)
        b_arr = bc[None, :, 0]                        # (1, R)
        c_arr = bc[None, :, 1]                        # (1, R)
        t_arr = c_arr * C - BURN + s_arr              # (STEPS, R)
        sym = np.where(
            (c_arr < 0) | (t_arr < 1) | (t_arr > TMAX),
            M,
            x_pad[np.broadcast_to(b_arr, t_arr.shape),
                  np.clip(t_arr, 1, TMAX)],
        )
        init_mask = (c_arr == 0) & (s_arr == BURN)
        sym = np.where(
            init_mask, x_pad[np.broadcast_to(b_arr, t_arr.shape), 0], sym
        )
        # big[s, r, n] -> eg[nw, p, sl, g, ic, rg]
        big = ehatT[sym]                              # (STEPS, R, N) bf16
        eg = big.reshape(NW, W, NGRP, RG, 2, 128).transpose(0, 5, 1, 2, 4, 3)
        eg_tensors.append(np.ascontiguousarray(eg))

    host = {
        "lam": lam,
        "T": T,
        "R": R,
        "cols": cols,
        "at": np.ascontiguousarray(at),
        "pi_t": np.ascontiguousarray(pi_t),
        "eg": eg_tensors,
    }
    return host


def _postprocess(zouts, host):
    """Combine per-core (1, 2, R) Zs/Ze into (B, 1) float32 log-probs."""
    lam, T, R, cols = host["lam"], host["T"], host["R"], host["cols"]
    Gsum = np.zeros(B, dtype=np.float64)
    L0 = np.zeros(B, dtype=np.float64)
    for core in range(NCORES):
        z = np.asarray(zouts[core], dtype=np.float64).reshape(2, R)
        Zs, Ze = z[0], z[1]
        with np.errstate(divide="ignore", invalid="ignore"):
            G = np.log(Ze) - np.log(Zs)
        for r, (b, c) in enumerate(cols[core]):
            if c < 0:
                continue
            Gsum[b] += G[r]
            if c == 0:
                L0[b] = np.log(Zs[r])
    L = L0 + Gsum - T * lam
    return L.reshape(B, 1).astype(np.float32)


def _make_in_maps(host):
    in_maps = []
    for core in range(NCORES):
        in_maps.append(
            {
                "at": host["at"],
                "pi0": host["pi_t"],
                "eg": host["eg"][core],
            }
        )
    return in_maps


def kernel(x, T, pi, trans, emis):
    host = _prep_inputs(x, T, pi, trans, emis)

    key = ("nc", host["R"])
    if key not in _CACHE:
        _CACHE[key] = _build_program(host["R"])
    nc = _CACHE[key]

    res = bass_utils.run_bass_kernel_spmd(
        nc, _make_in_maps(host), core_ids=list(range(NCORES))
    )
    zouts = [r["zout"] for r in res.results]
    return _postprocess(zouts, host)


def profile(inputs, tmpdir=None):
    """Run with trace=True; returns max-across-cores exec_time_ns."""
    host = _prep_inputs(**inputs)
    key = ("nc", host["R"])
    if key not in _CACHE:
        _CACHE[key] = _build_program(host["R"])
    nc = _CACHE[key]
    res = bass_utils.run_bass_kernel_spmd(
        nc,
        _make_in_maps(host),
        core_ids=list(range(NCORES)),
        trace=True,
        tmpdir=tmpdir,
    )
    return res.exec_time_ns
